# revision 11
# baseline (speedup 1.0000x reference)
"""Trainium2 Bass kernel for nn_BulkHamiltonian — v2 (9-plane f16 output).

Math (verified against the reference):
  phase1 = sqrt(3)*kx ; phase2 = (sqrt3/2)*kx + 1.5*ky
  s1 = sin(ph1), c1 = cos(ph1), s2 = sin(ph2), c2 = cos(ph2)
  Only 9 distinct k-dependent values exist per element; everything else
  in H is a constant or a duplicate of one of these:
    d1 = 0.75*s1          (col 5)        d2 = -d1   (col 33)
    d4 = -C34*s1          (cols 7,21)    d5 = -d4   (cols 35,49)
    d7 = 0.25*s1 + s2     (col 23)       d8 = -d7   (col 51)
    d0 = -0.75 - 0.75*c1  (cols 4,32)
    d3 = C34*(c1 - 1)     (cols 6,20,34,48)
    d6 = -0.25 - 0.25*c1 - c2  (cols 22,50)
  (cols are float32 indices inside the 64-float rows-4..7 slab of H)

Device computes the 9 planes in f16 (abs err ~5e-4 << 2e-2 gate); the
host only places constants and scatters/duplicates device values.

Per-core schedule (125056 padded elems = 128 partitions x 977 cols,
2 tiles of 389+588 cols; GPSIMD unused — its ops have ~14 ns/elem
completion latency and poison the critical path):
  DVE : x1, v, x2, cody(2w), |y| bitwise-abs(2w), d1, d4, d7,
        neg(3w) -> d2/d5/d8, d6 affine
  ACT : T=X*inv2pi+MAGIC (2w), Q=T-MAGIC (2w), Sin-a(2w), Sin-b(2w),
        d0, d3 (Copy)
  sync: input k prefetch, then per tile 2 output DMAs (planes 0:3
        right after Sin-a-dependent outputs, planes 3:9 at tile end)
  cos trick: sin(|y|-pi/2) = -cos(y) = mc; signs folded into constants.
"""

import sys
import types

import numpy as np

import concourse.bacc as bacc
import concourse.mybir as mybir
from concourse import bass_utils
from concourse.tile import TileContext


def _ensure_axon_hooks():
    try:
        import antenv.axon_hooks  # noqa: F401
        return
    except ImportError:
        pass
    hook = None
    try:
        from trn_agent_boot.trn_boot import _ntff_profile_via_ctypes

        hook = _ntff_profile_via_ctypes("/opt/axon/libaxon_pjrt.so")
    except Exception:
        hook = None
    mod = types.ModuleType("antenv.axon_hooks")
    mod.get_axon_ntff_profile_hook = lambda: hook
    mod.set_axon_ntff_profile_hook = lambda h: None
    try:
        import antenv

        sys.modules["antenv.axon_hooks"] = mod
        antenv.axon_hooks = mod
    except ImportError:
        sys.modules["antenv.axon_hooks"] = mod


_ensure_axon_hooks()

B_TOTAL = 1_000_000
N_CORES = 8
N_PER_CORE = B_TOTAL // N_CORES    # 125000
W_TOT = 977                        # ceil(125000/128)
N_PAD = 128 * W_TOT                # 125056
TILES = [(0, 389), (389, 588)]     # (col offset, width)

F32 = mybir.dt.float32
F16 = mybir.dt.float16

SQ3 = 1.7320508075688772
ISQ3 = 0.5773502691896258
C34 = 0.4330127018922193          # sqrt(3)/4
PIO2 = 1.5707963267948966
INV2PI = 0.15915494309189535
MAGIC = 12582912.0                # 1.5 * 2**23 round-to-nearest trick
TWOPI = 6.283185307179586

CW1 = float(np.float32(6.28125))
_r = TWOPI - CW1
_c2bits = np.float32(_r).view(np.uint32) & np.uint32(0xFFFFF000)
CW2 = float(_c2bits.view(np.float32))
CW3 = float(np.float32(_r - float(_c2bits.view(np.float32))))

# device plane -> float32 columns inside the 64-float rows-4..7 slab
PLANE_COLS = [
    [5],                # d1 = 0.75 s1
    [7, 21],            # d4 = -C34 s1
    [23],               # d7 = 0.25 s1 + s2
    [33],               # d2 = -d1
    [35, 49],           # d5 = -d4
    [51],               # d8 = -d7
    [4, 32],            # d0 = -0.75 - 0.75 c1
    [6, 20, 34, 48],    # d3 = C34 (c1 - 1)
    [22, 50],           # d6 = -0.25 - 0.25 c1 - c2
]

# constant template for one H row (128 f32 = 8x8 complex64)
TEMPLATE_ROW = np.zeros(128, dtype=np.float32)
for _rr in range(4):
    TEMPLATE_ROW[2 * (9 * _rr + 4)] = 1.0          # rows 0-3: [0 | I4]
for _c, _v in [(0, 1.5), (18, 1.5), (36, 1.5), (54, 1.5),
               (11, 0.2), (25, -0.2), (47, 0.2), (61, -0.2)]:
    TEMPLATE_ROW[64 + _c] = _v


def build_nc():
    nc = bacc.Bacc("TRN2", target_bir_lowering=False, debug=False,
                   enable_asserts=False)
    k_ap = nc.dram_tensor("k_in", [N_PAD, 2], F32, kind="ExternalInput").ap()
    o_ap = nc.dram_tensor("h_out", [128, 9 * W_TOT], F16,
                          kind="ExternalOutput").ap()
    kv = k_ap.rearrange("(p n) c -> p n c", p=128)

    k_all = nc.alloc_sbuf_tensor("k_all", [128, W_TOT, 2], F32).ap()
    bias_mpio2 = nc.alloc_sbuf_tensor("bias_mpio2", [128, 1], F32).ap()

    A = mybir.AluOpType
    AF = mybir.ActivationFunctionType

    with TileContext(nc) as tc:
        # prefetch all k tiles on the sync HWDGE queue (scalar's queue is
        # busy loading ACT tables then; GPSIMD/SWDGE has multi-us latency)
        for o, w in TILES:
            nc.sync.dma_start(k_all[:, o:o + w, :], kv[:, o:o + w, :])
        nc.vector.memset(bias_mpio2, -PIO2)
        # preload the Sin activation table during startup
        nc.scalar.activation(bias_mpio2, bias_mpio2, AF.Sin,
                             bias=0.0, scale=0.0)
        nc.vector.memset(bias_mpio2, -PIO2)

        with tc.tile_pool(name="work", bufs=2) as pool:
            for o, w in TILES:
                kx = k_all[:, o:o + w, 0]
                ky = k_all[:, o:o + w, 1]

                X = pool.tile([128, 2, w], F32, tag="X", name="X")
                T = pool.tile([128, 2, w], F32, tag="T", name="T")
                Qt = pool.tile([128, 2, w], F32, tag="Q", name="Q")
                P = pool.tile([128, 4, w], F32, tag="P", name="P")
                S = pool.tile([128, 4, w], F16, tag="S", name="S")
                vv = pool.tile([128, w], F32, tag="v", name="v")
                O = pool.tile([128, 9, w], F16, tag="O", name="O")

                # phases
                nc.vector.tensor_scalar(X[:, 0, :], kx, SQ3, None, A.mult)
                nc.vector.scalar_tensor_tensor(vv, kx, ISQ3, ky, A.mult, A.add)
                nc.vector.tensor_scalar(X[:, 1, :], vv, 1.5, None, A.mult)

                X2 = X.rearrange("p a w -> p (a w)")
                T2 = T.rearrange("p a w -> p (a w)")
                Q2 = Qt.rearrange("p a w -> p (a w)")
                nc.scalar.activation(T2, X2, AF.Copy, bias=MAGIC, scale=INV2PI)
                nc.scalar.activation(Q2, T2, AF.Copy, bias=-MAGIC, scale=1.0)

                Y2 = P[:, 0:2, :].rearrange("p a w -> p (a w)")
                A2 = P[:, 2:4, :].rearrange("p a w -> p (a w)")
                nc.vector.cody_waite_cascade(Y2, X2, Q2, CW1, CW2, CW3)
                # |y| via sign-bit clear; the -pi/2 shift folds into Sin bias
                nc.vector.tensor_scalar(
                    A2.bitcast(mybir.dt.uint32), Y2.bitcast(mybir.dt.uint32),
                    0x7FFFFFFF, None, A.bitwise_and)

                # S = [s1, s2, mc1, mc2] (mc = -cos = sin(|y| - pi/2)), f16
                S2a = S[:, 0:2, :].rearrange("p a w -> p (a w)")
                S2b = S[:, 2:4, :].rearrange("p a w -> p (a w)")
                nc.scalar.activation(S2a, Y2, AF.Sin)
                nc.scalar.activation(S2b, A2, AF.Sin, bias=bias_mpio2, scale=1.0)

                s1 = S[:, 0, :]
                s2 = S[:, 1, :]
                mc1 = S[:, 2, :]
                mc2 = S[:, 3, :]

                # outputs; planes 0-2 only need Sin-a -> DMA them early
                nc.vector.tensor_scalar(O[:, 0, :], s1, 0.75, None, A.mult)
                nc.vector.tensor_scalar(O[:, 1, :], s1, -C34, None, A.mult)
                nc.vector.scalar_tensor_tensor(O[:, 2, :], s1, 0.25, s2,
                                               A.mult, A.add)
                nc.sync.dma_start(
                    o_ap[:, 9 * o: 9 * o + 3 * w],
                    O[:, 0:3, :].rearrange("p j w -> p (j w)"),
                )
                nc.vector.tensor_scalar(
                    O[:, 3:6, :].rearrange("p a w -> p (a w)"),
                    O[:, 0:3, :].rearrange("p a w -> p (a w)"),
                    -1.0, None, A.mult)
                nc.sync.dma_start(
                    o_ap[:, 9 * o + 3 * w: 9 * o + 6 * w],
                    O[:, 3:6, :].rearrange("p j w -> p (j w)"),
                )
                nc.scalar.activation(O[:, 6, :], mc1, AF.Copy,
                                     bias=-0.75, scale=0.75)
                nc.scalar.activation(O[:, 7, :], mc1, AF.Copy,
                                     bias=-C34, scale=-C34)
                nc.vector.affine_then_add(O[:, 8, :], mc1, mc2,
                                          scale=0.25, bias=-0.25)
                nc.sync.dma_start(
                    o_ap[:, 9 * o + 6 * w: 9 * (o + w)],
                    O[:, 6:9, :].rearrange("p j w -> p (j w)"),
                )
    nc.compile()
    return nc


_CACHE = {}


def _get_nc():
    if "nc" not in _CACHE:
        _CACHE["nc"] = build_nc()
    return _CACHE["nc"]


def run_spmd(k_flat, **kwargs):
    """k_flat: [B_TOTAL, 2] float32. Returns (per-core h_out arrays, res)."""
    shards = np.zeros((N_CORES, N_PAD, 2), dtype=np.float32)
    shards[:, :N_PER_CORE, :] = np.ascontiguousarray(k_flat).reshape(
        N_CORES, N_PER_CORE, 2)
    nc = _get_nc()
    in_maps = [{"k_in": shards[i]} for i in range(N_CORES)]
    res = bass_utils.run_bass_kernel_spmd(
        nc, in_maps, core_ids=list(range(N_CORES)), **kwargs
    )
    return [res.results[i]["h_out"] for i in range(N_CORES)], res


def kernel(k):
    k = np.asarray(k, dtype=np.float32).reshape(B_TOTAL, 2)
    shards, _ = run_spmd(k)

    # decode per-core plane-major tiles -> [B_TOTAL, 9] (f16, no math)
    d_all = np.empty((N_CORES, 128, W_TOT, 9), dtype=np.float16)
    for i in range(N_CORES):
        out = shards[i]
        for o, w in TILES:
            blk = out[:, 9 * o: 9 * (o + w)].reshape(128, 9, w)
            d_all[i, :, o:o + w, :] = blk.transpose(0, 2, 1)
    d_flat = d_all.reshape(N_CORES, N_PAD, 9)

    H = np.empty((B_TOTAL, 8, 8), dtype=np.complex64)
    Hf = H.view(np.float32).reshape(B_TOTAL, 128)
    CH = 1 << 17
    for i in range(N_CORES):
        base = i * N_PER_CORE
        for a in range(0, N_PER_CORE, CH):
            b = min(a + CH, N_PER_CORE)
            dc = d_flat[i, a:b].astype(np.float32)
            Hf[base + a: base + b] = TEMPLATE_ROW
            for pl, cols in enumerate(PLANE_COLS):
                for c in cols:
                    Hf[base + a: base + b, 64 + c] = dc[:, pl]
    return H


# revision 12
# speedup vs baseline: 1.0455x; 1.0455x over previous
"""Trainium2 Bass kernel for nn_BulkHamiltonian — v2 (9-plane f16 output).

Math (verified against the reference):
  phase1 = sqrt(3)*kx ; phase2 = (sqrt3/2)*kx + 1.5*ky
  s1 = sin(ph1), c1 = cos(ph1), s2 = sin(ph2), c2 = cos(ph2)
  Only 9 distinct k-dependent values exist per element; everything else
  in H is a constant or a duplicate of one of these:
    d1 = 0.75*s1          (col 5)        d2 = -d1   (col 33)
    d4 = -C34*s1          (cols 7,21)    d5 = -d4   (cols 35,49)
    d7 = 0.25*s1 + s2     (col 23)       d8 = -d7   (col 51)
    d0 = -0.75 - 0.75*c1  (cols 4,32)
    d3 = C34*(c1 - 1)     (cols 6,20,34,48)
    d6 = -0.25 - 0.25*c1 - c2  (cols 22,50)
  (cols are float32 indices inside the 64-float rows-4..7 slab of H)

Device computes the 9 planes in f16 (abs err ~5e-4 << 2e-2 gate); the
host only places constants and scatters/duplicates device values.

Per-core schedule (125056 padded elems = 128 partitions x 977 cols,
2 tiles of 389+588 cols; GPSIMD unused — its ops have ~14 ns/elem
completion latency and poison the critical path):
  DVE : x1, v, x2, cody(2w), |y| bitwise-abs(2w), d1, d4, d7,
        neg(3w) -> d2/d5/d8, d6 affine
  ACT : T=X*inv2pi+MAGIC (2w), Q=T-MAGIC (2w), Sin-a(2w), Sin-b(2w),
        d0, d3 (Copy)
  sync: input k prefetch, then per tile 2 output DMAs (planes 0:3
        right after Sin-a-dependent outputs, planes 3:9 at tile end)
  cos trick: sin(|y|-pi/2) = -cos(y) = mc; signs folded into constants.
"""

import sys
import types

import numpy as np

import concourse.bacc as bacc
import concourse.mybir as mybir
from concourse import bass_utils
from concourse.tile import TileContext


def _ensure_axon_hooks():
    try:
        import antenv.axon_hooks  # noqa: F401
        return
    except ImportError:
        pass
    hook = None
    try:
        from trn_agent_boot.trn_boot import _ntff_profile_via_ctypes

        hook = _ntff_profile_via_ctypes("/opt/axon/libaxon_pjrt.so")
    except Exception:
        hook = None
    mod = types.ModuleType("antenv.axon_hooks")
    mod.get_axon_ntff_profile_hook = lambda: hook
    mod.set_axon_ntff_profile_hook = lambda h: None
    try:
        import antenv

        sys.modules["antenv.axon_hooks"] = mod
        antenv.axon_hooks = mod
    except ImportError:
        sys.modules["antenv.axon_hooks"] = mod


_ensure_axon_hooks()

B_TOTAL = 1_000_000
N_CORES = 8
N_PER_CORE = B_TOTAL // N_CORES    # 125000
W_TOT = 977                        # ceil(125000/128)
N_PAD = 128 * W_TOT                # 125056
TILES = [(0, 389), (389, 588)]     # (col offset, width)

F32 = mybir.dt.float32
F16 = mybir.dt.float16

SQ3 = 1.7320508075688772
ISQ3 = 0.5773502691896258
C34 = 0.4330127018922193          # sqrt(3)/4
PIO2 = 1.5707963267948966
INV2PI = 0.15915494309189535
MAGIC = 12582912.0                # 1.5 * 2**23 round-to-nearest trick
TWOPI = 6.283185307179586

CW1 = float(np.float32(6.28125))
_r = TWOPI - CW1
_c2bits = np.float32(_r).view(np.uint32) & np.uint32(0xFFFFF000)
CW2 = float(_c2bits.view(np.float32))
CW3 = float(np.float32(_r - float(_c2bits.view(np.float32))))

# device plane -> float32 columns inside the 64-float rows-4..7 slab
PLANE_COLS = [
    [5],                # d1 = 0.75 s1
    [7, 21],            # d4 = -C34 s1
    [23],               # d7 = 0.25 s1 + s2
    [33],               # d2 = -d1
    [35, 49],           # d5 = -d4
    [51],               # d8 = -d7
    [4, 32],            # d0 = -0.75 - 0.75 c1
    [6, 20, 34, 48],    # d3 = C34 (c1 - 1)
    [22, 50],           # d6 = -0.25 - 0.25 c1 - c2
]

# constant template for one H row (128 f32 = 8x8 complex64)
TEMPLATE_ROW = np.zeros(128, dtype=np.float32)
for _rr in range(4):
    TEMPLATE_ROW[2 * (9 * _rr + 4)] = 1.0          # rows 0-3: [0 | I4]
for _c, _v in [(0, 1.5), (18, 1.5), (36, 1.5), (54, 1.5),
               (11, 0.2), (25, -0.2), (47, 0.2), (61, -0.2)]:
    TEMPLATE_ROW[64 + _c] = _v


def build_nc():
    nc = bacc.Bacc("TRN2", target_bir_lowering=False, debug=False,
                   enable_asserts=False)
    k_ap = nc.dram_tensor("k_in", [N_PAD, 2], F32, kind="ExternalInput").ap()
    o_ap = nc.dram_tensor("h_out", [128, 9 * W_TOT], F16,
                          kind="ExternalOutput").ap()
    kv = k_ap.rearrange("(p n) c -> p n c", p=128)

    k_all = nc.alloc_sbuf_tensor("k_all", [128, W_TOT, 2], F32).ap()
    bias_mpio2 = nc.alloc_sbuf_tensor("bias_mpio2", [128, 1], F32).ap()

    A = mybir.AluOpType
    AF = mybir.ActivationFunctionType

    with TileContext(nc) as tc:
        # prefetch all k tiles on the sync HWDGE queue (scalar's queue is
        # busy loading ACT tables then; GPSIMD/SWDGE has multi-us latency)
        for o, w in TILES:
            nc.sync.dma_start(k_all[:, o:o + w, :], kv[:, o:o + w, :])
        nc.vector.memset(bias_mpio2, -PIO2)
        # preload the Sin activation table during startup
        nc.scalar.activation(bias_mpio2, bias_mpio2, AF.Sin,
                             bias=0.0, scale=0.0)
        nc.vector.memset(bias_mpio2, -PIO2)

        with tc.tile_pool(name="work", bufs=2) as pool:
            for o, w in TILES:
                kx = k_all[:, o:o + w, 0]
                ky = k_all[:, o:o + w, 1]

                X = pool.tile([128, 2, w], F32, tag="X", name="X")
                T = pool.tile([128, 2, w], F32, tag="T", name="T")
                Qt = pool.tile([128, 2, w], F32, tag="Q", name="Q")
                P = pool.tile([128, 4, w], F32, tag="P", name="P")
                S = pool.tile([128, 4, w], F16, tag="S", name="S")
                vv = pool.tile([128, w], F32, tag="v", name="v")
                O = pool.tile([128, 9, w], F16, tag="O", name="O")

                # phases
                nc.vector.tensor_scalar(X[:, 0, :], kx, SQ3, None, A.mult)
                nc.vector.scalar_tensor_tensor(vv, kx, ISQ3, ky, A.mult, A.add)
                nc.vector.tensor_scalar(X[:, 1, :], vv, 1.5, None, A.mult)

                X2 = X.rearrange("p a w -> p (a w)")
                T2 = T.rearrange("p a w -> p (a w)")
                Q2 = Qt.rearrange("p a w -> p (a w)")
                nc.scalar.activation(T2, X2, AF.Copy, bias=MAGIC, scale=INV2PI)
                nc.scalar.activation(Q2, T2, AF.Copy, bias=-MAGIC, scale=1.0)

                Y2 = P[:, 0:2, :].rearrange("p a w -> p (a w)")
                A2 = P[:, 2:4, :].rearrange("p a w -> p (a w)")
                nc.vector.cody_waite_cascade(Y2, X2, Q2, CW1, CW2, CW3)
                # |y| via sign-bit clear; the -pi/2 shift folds into Sin bias
                nc.vector.tensor_scalar(
                    A2.bitcast(mybir.dt.uint32), Y2.bitcast(mybir.dt.uint32),
                    0x7FFFFFFF, None, A.bitwise_and)

                # S = [s1, s2, mc1, mc2] (mc = -cos = sin(|y| - pi/2)), f16
                S2a = S[:, 0:2, :].rearrange("p a w -> p (a w)")
                S2b = S[:, 2:4, :].rearrange("p a w -> p (a w)")
                nc.scalar.activation(S2a, Y2, AF.Sin)
                nc.scalar.activation(S2b, A2, AF.Sin, bias=bias_mpio2, scale=1.0)

                s1 = S[:, 0, :]
                s2 = S[:, 1, :]
                mc1 = S[:, 2, :]
                mc2 = S[:, 3, :]

                # outputs; planes 0-2 only need Sin-a -> DMA them early
                nc.vector.tensor_scalar(O[:, 0, :], s1, 0.75, None, A.mult)
                nc.vector.tensor_scalar(O[:, 1, :], s1, -C34, None, A.mult)
                nc.vector.scalar_tensor_tensor(O[:, 2, :], s1, 0.25, s2,
                                               A.mult, A.add)
                nc.sync.dma_start(
                    o_ap[:, 9 * o: 9 * o + 3 * w],
                    O[:, 0:3, :].rearrange("p j w -> p (j w)"),
                )
                nc.vector.tensor_scalar(
                    O[:, 3:6, :].rearrange("p a w -> p (a w)"),
                    O[:, 0:3, :].rearrange("p a w -> p (a w)"),
                    -1.0, None, A.mult)
                nc.scalar.activation(O[:, 6, :], mc1, AF.Copy,
                                     bias=-0.75, scale=0.75)
                nc.scalar.activation(O[:, 7, :], mc1, AF.Copy,
                                     bias=-C34, scale=-C34)
                nc.vector.affine_then_add(O[:, 8, :], mc1, mc2,
                                          scale=0.25, bias=-0.25)
                nc.sync.dma_start(
                    o_ap[:, 9 * o + 3 * w: 9 * (o + w)],
                    O[:, 3:9, :].rearrange("p j w -> p (j w)"),
                )
    nc.compile()
    return nc


_CACHE = {}


def _get_nc():
    if "nc" not in _CACHE:
        _CACHE["nc"] = build_nc()
    return _CACHE["nc"]


def run_spmd(k_flat, **kwargs):
    """k_flat: [B_TOTAL, 2] float32. Returns (per-core h_out arrays, res)."""
    shards = np.zeros((N_CORES, N_PAD, 2), dtype=np.float32)
    shards[:, :N_PER_CORE, :] = np.ascontiguousarray(k_flat).reshape(
        N_CORES, N_PER_CORE, 2)
    nc = _get_nc()
    in_maps = [{"k_in": shards[i]} for i in range(N_CORES)]
    res = bass_utils.run_bass_kernel_spmd(
        nc, in_maps, core_ids=list(range(N_CORES)), **kwargs
    )
    return [res.results[i]["h_out"] for i in range(N_CORES)], res


def kernel(k):
    k = np.asarray(k, dtype=np.float32).reshape(B_TOTAL, 2)
    shards, _ = run_spmd(k)

    # decode per-core plane-major tiles -> [B_TOTAL, 9] (f16, no math)
    d_all = np.empty((N_CORES, 128, W_TOT, 9), dtype=np.float16)
    for i in range(N_CORES):
        out = shards[i]
        for o, w in TILES:
            blk = out[:, 9 * o: 9 * (o + w)].reshape(128, 9, w)
            d_all[i, :, o:o + w, :] = blk.transpose(0, 2, 1)
    d_flat = d_all.reshape(N_CORES, N_PAD, 9)

    H = np.empty((B_TOTAL, 8, 8), dtype=np.complex64)
    Hf = H.view(np.float32).reshape(B_TOTAL, 128)
    CH = 1 << 17
    for i in range(N_CORES):
        base = i * N_PER_CORE
        for a in range(0, N_PER_CORE, CH):
            b = min(a + CH, N_PER_CORE)
            dc = d_flat[i, a:b].astype(np.float32)
            Hf[base + a: base + b] = TEMPLATE_ROW
            for pl, cols in enumerate(PLANE_COLS):
                for c in cols:
                    Hf[base + a: base + b, 64 + c] = dc[:, pl]
    return H
